# revision 14
# baseline (speedup 1.0000x reference)
"""Trainium2 Bass kernel for the two-qubit weak-measurement Euler SDE sampler.

Math reformulation (exactly equivalent to the reference, validated to 1e-15):
  rho = X + iY (Hermitian, X sym / Y antisym real 4x4), flattened c = 4*i+j.
  C1, C2, HZ are diagonal, so the dissipator / measurement / detuning terms are
  elementwise masks over c; only the HX commutator mixes components and is
  applied as a constant 32x32 matrix on the Omega-prescaled state qp = dt*Omega*r.

  Per Euler step (g = sqge*dw1, h = sqge*dw2, per trajectory):
    t1 = 2*(X00+X11-X22-X33), t2 = 2*(X00-X11+X22-X33)
    W  = g*A1 + h*A2 - (g*t1 + h*t2)          (noise multiplier, via PE)
    X' = X*(W + 1 + dt*G*D) + dt*(eps*Z.Y + Omega*MHX(Y))
    Y' = Y*(W + 1 + dt*G*D) - dt*(eps*Z.X + Omega*MHX(X))

Device layout (per core, 1024 trajectories = 16 traj-copies x 64 batch):
  partitions m = xy*64 + u*16 + c   (xy: 0=X 1=Y, u: sub-block of 256 traj)
  free n = 0..255; local traj j = u*256 + n; batch b = n % 64.

Sharding: trajectory-copy axis across 8 cores (t in [16k, 16k+16)), final mean
reduced on host.
"""

import os
import sys

for _p in ("/opt/trn_rl_repo", "/opt/pypackages"):
    if _p not in sys.path:
        sys.path.append(_p)

import numpy as np
import concourse.bass as bass
import concourse.bacc as bacc
import concourse.mybir as mybir
import concourse.tile as tile
from concourse.alu_op_type import AluOpType
from concourse.bass_utils import run_bass_kernel_spmd

B, NT, NSTEP = 64, 128, 255
DT = 2.0 ** -8
NCORES = 8
TPC = NT // NCORES        # 16 trajectory copies per core
NLOC = TPC * B            # 1024 trajectories per core
SUBS, COLS = 4, 256

F32 = mybir.dt.float32
F32R = mybir.dt.float32r
F16 = mybir.dt.float16

_c1 = np.array([1, 1, -1, -1], np.float64)
_c2 = np.array([1, -1, 1, -1], np.float64)
_z = np.array([1, 0, 0, -1], np.float64)
_sx = np.array([[0, 1], [1, 0]], np.float64)
_HX = 0.5 * (np.kron(_sx, np.eye(2)) + np.kron(np.eye(2), _sx))

_A1 = (_c1[:, None] + _c1[None, :]).ravel()
_A2 = (_c2[:, None] + _c2[None, :]).ravel()
_D = (_c1[:, None] * _c1[None, :] + _c2[:, None] * _c2[None, :] - 2.0).ravel()
_Z = (_z[:, None] - _z[None, :]).ravel()

# MHX[c, c'] : (HX M - M HX)_c = sum_c' MHX[c,c'] M_c'
_MHX = np.zeros((16, 16), np.float64)
for _i in range(4):
    for _j in range(4):
        for _ii in range(4):
            for _jj in range(4):
                _v = _HX[_i, _ii] * (_j == _jj) - (_i == _ii) * _HX[_jj, _j]
                _MHX[4 * _i + _j, 4 * _ii + _jj] = _v


def _meas_ops():
    sx = np.array([[0, 1], [1, 0]], np.complex128)
    sy = np.array([[0, -1j], [1j, 0]], np.complex128)
    sz = np.array([[1, 0], [0, -1]], np.complex128)
    I2 = np.eye(2, dtype=np.complex128)
    paulis = [sx, sy, sz]
    ops = []
    for A in paulis:
        for Bm in paulis:
            for sa in (1, -1):
                for sb in (1, -1):
                    ops.append(np.kron((I2 + sa * A) / 2, (I2 + sb * Bm) / 2))
    for A in paulis:
        ops.append(np.kron((I2 + A) / 2, I2))
    for Bm in paulis:
        ops.append(np.kron(I2, (I2 + Bm) / 2))
    return np.stack(ops)  # [42, 4, 4]


_MEAS = _meas_ops()


def _part(xy, u, c):
    return xy * 64 + u * 16 + c


def _host_weights(eps, gamma):
    """Constant lhsT matrices. lhsT[k, m]: out[m,n] = sum_k lhsT[k,m]*rhs[k,n]."""
    zeps = np.zeros((128, 128), np.float64)
    mxm = np.zeros((128, 128), np.float64)
    for u in range(SUBS):
        for c in range(16):
            zeps[_part(1, u, c), _part(0, u, c)] = DT * eps * _Z[c]
            zeps[_part(0, u, c), _part(1, u, c)] = -DT * eps * _Z[c]
            for cp in range(16):
                if _MHX[c, cp] != 0.0:
                    mxm[_part(1, u, cp), _part(0, u, c)] = _MHX[c, cp]
                    mxm[_part(0, u, cp), _part(1, u, c)] = -_MHX[c, cp]
    fmat = np.zeros((128, 8), np.float64)
    for u in range(SUBS):
        for i in range(4):
            fmat[_part(0, u, 5 * i), u] = 2.0 * _c1[i]
            fmat[_part(0, u, 5 * i), 4 + u] = 2.0 * _c2[i]
    selgh = np.zeros((8, 128), np.float64)
    negsel = np.zeros((8, 128), np.float64)
    for xy in range(2):
        for u in range(SUBS):
            for c in range(16):
                m = _part(xy, u, c)
                selgh[u, m] = _A1[c]
                selgh[4 + u, m] = _A2[c]
                negsel[u, m] = -1.0
                negsel[4 + u, m] = -1.0
    wdg = np.zeros((128, 1), np.float64)
    for xy in range(2):
        for u in range(SUBS):
            for c in range(16):
                wdg[_part(xy, u, c), 0] = 1.0 + DT * gamma * _D[c]
    summ = np.zeros((128, 32), np.float64)
    for xy in range(2):
        for u in range(SUBS):
            for c in range(16):
                summ[_part(xy, u, c), xy * 16 + c] = 1.0
    fz = zeps @ fmat          # lhsT composition: (F o Zeps)[k, m']
    fm = mxm @ fmat
    f16, f32 = np.float16, np.float32
    return dict(
        zeps_w=zeps.astype(f16), mxm_w=mxm.astype(f16), f_w=fmat.astype(f16),
        fz_w=fz.astype(f16), fm_w=fm.astype(f16), selgh_w=selgh.astype(f16),
        negsel_w=negsel.astype(f16), wdg_w=wdg.astype(f32),
        summ_w=summ.astype(f32))


def _round_f32r(a):
    """Round float32 array to fp32r (11 mantissa bits, RNE) — what the PE
    expects of any tensor feeding a float32r matmul."""
    bits = np.ascontiguousarray(a, np.float32).view(np.uint32).copy()
    keep = (bits >> 12) & 1
    bits += 0x7FF + keep
    bits &= np.uint32(0xFFFFF000)
    return bits.view(np.float32)


_PROGRAM_CACHE = {}


def _build_program():
    if "nc" in _PROGRAM_CACHE:
        return _PROGRAM_CACHE["nc"]
    nc = bacc.Bacc("TRN2", target_bir_lowering=False, debug=False,
                   enable_asserts=False, num_devices=NCORES)

    noise_d = nc.dram_tensor("noise", [NSTEP, 8, COLS], F16, kind="ExternalInput")
    r0_d = nc.dram_tensor("r0", [128, COLS], F32, kind="ExternalInput")
    dtom_d = nc.dram_tensor("dtom", [128, COLS], F32, kind="ExternalInput")
    wdims = {"zeps_w": ([128, 128], F16), "mxm_w": ([128, 128], F16),
             "f_w": ([128, 8], F16), "fz_w": ([128, 8], F16),
             "fm_w": ([128, 8], F16), "selgh_w": ([8, 128], F16),
             "negsel_w": ([8, 128], F16), "wdg_w": ([128, 1], F32),
             "summ_w": ([128, 32], F32)}
    wdram = {n: nc.dram_tensor(n, shp, dt, kind="ExternalInput")
             for n, (shp, dt) in wdims.items()}
    out_d = nc.dram_tensor("partial", [32, B], F32, kind="ExternalOutput")

    MULT, ADD = AluOpType.mult, AluOpType.add

    with tile.TileContext(nc) as tc:
        with (
            tc.tile_pool(name="const", bufs=1) as cpool,
            tc.tile_pool(name="state", bufs=2) as spool,
            tc.tile_pool(name="work", bufs=2) as wpool,
            tc.tile_pool(name="gh", bufs=6) as ghpool,
            tc.tile_pool(name="psum", bufs=2, space="PSUM") as ppool,
        ):
            w = {}
            for n, (shp, dt) in wdims.items():
                w[n] = cpool.tile(shp, dt, tag=n, name=n + "_sb")
                nc.sync.dma_start(w[n][:], wdram[n].ap())
            dtom = cpool.tile([128, COLS], F32, tag="dtom")
            nc.sync.dma_start(dtom[:], dtom_d.ap())

            r = spool.tile([128, COLS], F32, tag="r")
            nc.sync.dma_start(r[:], r0_d.ap())
            r16 = spool.tile([128, COLS], F16, tag="r16")
            nc.scalar.copy(r16[:], r[:])
            qp16 = spool.tile([128, COLS], F16, tag="qp16")
            nc.gpsimd.tensor_mul(qp16[:], r[:], dtom[:])

            # step-0 prep: T(0) = F @ r(0) (full fp32), p8(0), W(0)
            gh = ghpool.tile([8, COLS], F16, tag="gh")
            nc.sync.dma_start(gh[:], noise_d.ap()[0, :, :])
            psum_t = ppool.tile([8, COLS], F32, tag="T")
            nc.tensor.matmul(psum_t[:], w["f_w"][:], r16[:], start=True, stop=True)
            p8 = wpool.tile([8, COLS], F16, tag="p8")
            nc.vector.tensor_tensor(p8[:], gh[:], psum_t[:], op=MULT)
            psum_w = ppool.tile([128, COLS], F32, tag="W")
            nc.tensor.matmul(psum_w[:], w["selgh_w"][:], gh[:],
                             start=True, stop=False)
            nc.tensor.matmul(psum_w[:], w["negsel_w"][:], p8[:],
                             start=False, stop=True)

            for k in range(NSTEP):
                last = k == NSTEP - 1
                if not last:
                    gh_nx = ghpool.tile([8, COLS], F16, tag="gh")
                    nc.sync.dma_start(gh_nx[:], noise_d.ap()[k + 1, :, :])

                psum_a = ppool.tile([128, COLS], F32, tag="A")
                nc.tensor.matmul(psum_a[:], w["zeps_w"][:], r16[:],
                                 start=True, stop=False)
                nc.tensor.matmul(psum_a[:], w["mxm_w"][:], qp16[:],
                                 start=False, stop=True)
                if not last:
                    psum_t_nx = ppool.tile([8, COLS], F32, tag="T")
                    nc.tensor.matmul(psum_t_nx[:], w["fz_w"][:], r16[:],
                                     start=True, stop=False)
                    nc.tensor.matmul(psum_t_nx[:], w["fm_w"][:], qp16[:],
                                     start=False, stop=False)

                v = wpool.tile([128, COLS], F32, tag="v")
                nc.vector.scalar_tensor_tensor(v[:], psum_w[:], w["wdg_w"][:],
                                               r[:], op0=ADD, op1=MULT)
                if not last:
                    v16 = wpool.tile([128, COLS], F16, tag="v16")
                    nc.gpsimd.tensor_copy(v16[:], v[:])
                    nc.tensor.matmul(psum_t_nx[:], w["f_w"][:], v16[:],
                                     start=False, stop=True)
                r_nx = spool.tile([128, COLS], F32, tag="r")
                nc.vector.tensor_tensor(r_nx[:], v[:], psum_a[:], op=ADD)

                if not last:
                    r16_nx = spool.tile([128, COLS], F16, tag="r16")
                    nc.scalar.copy(r16_nx[:], r_nx[:])
                    qp16_nx = spool.tile([128, COLS], F16, tag="qp16")
                    nc.gpsimd.tensor_mul(qp16_nx[:], r_nx[:], dtom[:])
                    p8_nx = wpool.tile([8, COLS], F16, tag="p8")
                    nc.vector.tensor_tensor(p8_nx[:], gh_nx[:], psum_t_nx[:],
                                            op=MULT)
                    psum_w_nx = ppool.tile([128, COLS], F32, tag="W")
                    nc.tensor.matmul(psum_w_nx[:], w["selgh_w"][:], gh_nx[:],
                                     start=True, stop=False)
                    nc.tensor.matmul(psum_w_nx[:], w["negsel_w"][:], p8_nx[:],
                                     start=False, stop=True)
                    r, r16, qp16 = r_nx, r16_nx, qp16_nx
                    psum_w, psum_t = psum_w_nx, psum_t_nx
                else:
                    r = r_nx

            psum_o = ppool.tile([32, COLS], F32, tag="T")
            nc.tensor.matmul(psum_o[:], w["summ_w"][:], r[:],
                             start=True, stop=True)
            o1 = wpool.tile([32, COLS], F32, tag="o1")
            nc.vector.tensor_copy(o1[:], psum_o[:])
            o2 = wpool.tile([32, 128], F32, tag="o2")
            nc.vector.tensor_tensor(o2[:], o1[:, 0:128], o1[:, 128:256], op=ADD)
            o3 = wpool.tile([32, B], F32, tag="o3")
            nc.vector.tensor_tensor(o3[:], o2[:, 0:B], o2[:, B:128], op=ADD)
            nc.sync.dma_start(out_d.ap(), o3[:])

    nc.compile()
    _PROGRAM_CACHE["nc"] = nc
    return nc


def _host_prep(inputs, params, wvec, rho0):
    inputs = np.asarray(inputs, np.float64)
    params = np.asarray(params, np.float64)
    wvec64 = np.asarray(wvec, np.float64)
    rho0 = np.asarray(rho0)

    omega = inputs[:, 0] + 1e-8
    eps, gamma, eta = params[1], params[2], params[3]
    sqge = np.sqrt(max(gamma * eta, 0.0))

    weights = _host_weights(eps, gamma)

    dtom = np.tile((DT * omega).astype(np.float32)[None, :], (128, SUBS))
    # column n -> batch b = n % 64 (j = u*256 + n, 256 % 64 == 0)

    x0 = np.real(rho0).astype(np.float64).ravel()
    y0 = np.imag(rho0).astype(np.float64).ravel()
    r0 = np.zeros((128, COLS), np.float32)
    for u in range(SUBS):
        r0[_part(0, u, 0):_part(0, u, 0) + 16, :] = x0[:, None]
        r0[_part(1, u, 0):_part(1, u, 0) + 16, :] = y0[:, None]

    noise_cores = []
    for core in range(NCORES):
        wv = wvec64[core * NLOC:(core + 1) * NLOC]          # [1024, 255, 2]
        wv = (sqge * wv).astype(np.float16)
        wv = wv.reshape(SUBS, COLS, NSTEP, 2)                # [u, n, k, ch]
        noise = np.empty((NSTEP, 8, COLS), np.float16)
        noise[:, 0:4, :] = wv[:, :, :, 0].transpose(2, 0, 1)
        noise[:, 4:8, :] = wv[:, :, :, 1].transpose(2, 0, 1)
        noise_cores.append(np.ascontiguousarray(noise))

    shared = dict(weights, r0=r0, dtom=dtom)
    in_maps = [dict(shared, noise=noise_cores[c]) for c in range(NCORES)]
    return in_maps, inputs


def _postprocess(partials, inputs64):
    """partials: list of 8 arrays [32, 64] -> (out [64,43] f64, rho_mean c128)."""
    acc = np.zeros((32, B), np.float64)
    for p in partials:
        acc += np.asarray(p, np.float64)
    acc /= NT
    xbar = acc[0:16]            # [16, 64], c-major
    ybar = acc[16:32]
    rho_mean = (xbar + 1j * ybar).T.reshape(B, 4, 4)
    probs = np.einsum("kij,bji->bk", _MEAS, rho_mean).real
    probs = np.clip(probs, 0.0, 1.0)
    out = np.concatenate([probs, inputs64], axis=1)
    return out, rho_mean


TRACE = bool(int(os.environ.get("KERNEL_TRACE", "0")))
LAST_RESULTS = None


def kernel(inputs, params, wvec, rho0):
    global LAST_RESULTS
    in_maps, inputs64 = _host_prep(inputs, params, wvec, rho0)
    nc = _build_program()
    res = run_bass_kernel_spmd(nc, in_maps, core_ids=list(range(NCORES)),
                               trace=TRACE)
    LAST_RESULTS = res
    partials = [res.results[c]["partial"] for c in range(NCORES)]
    return _postprocess(partials, inputs64)


# revision 15
# speedup vs baseline: 1.2123x; 1.2123x over previous
"""Trainium2 Bass kernel for the two-qubit weak-measurement Euler SDE sampler.

Math reformulation (exactly equivalent to the reference, validated to 1e-15):
  rho = X + iY (Hermitian, X sym / Y antisym real 4x4), flattened c = 4*i+j.
  C1, C2, HZ are diagonal, so the dissipator / measurement / detuning terms are
  elementwise masks over c; only the HX commutator mixes components and is
  applied as a constant 32x32 matrix on the Omega-prescaled state qp = dt*Omega*r.

  Per Euler step (g = sqge*dw1, h = sqge*dw2, per trajectory):
    t1 = 2*(X00+X11-X22-X33), t2 = 2*(X00-X11+X22-X33)
    W  = g*A1 + h*A2 - (g*t1 + h*t2)          (noise multiplier, via PE)
    X' = X*(W + 1 + dt*G*D) + dt*(eps*Z.Y + Omega*MHX(Y))
    Y' = Y*(W + 1 + dt*G*D) - dt*(eps*Z.X + Omega*MHX(X))

Device layout (per core, 1024 trajectories = 16 traj-copies x 64 batch):
  partitions m = xy*64 + u*16 + c   (xy: 0=X 1=Y, u: sub-block of 256 traj)
  free n = 0..255; local traj j = u*256 + n; batch b = n % 64.

Sharding: trajectory-copy axis across 8 cores (t in [16k, 16k+16)), final mean
reduced on host.
"""

import os
import sys

for _p in ("/opt/trn_rl_repo", "/opt/pypackages"):
    if _p not in sys.path:
        sys.path.append(_p)

import numpy as np
import concourse.bass as bass
import concourse.bacc as bacc
import concourse.mybir as mybir
import concourse.tile as tile
from concourse.alu_op_type import AluOpType
from concourse.bass_utils import run_bass_kernel_spmd

B, NT, NSTEP = 64, 128, 255
DT = 2.0 ** -8
NCORES = 8
TPC = NT // NCORES        # 16 trajectory copies per core
NLOC = TPC * B            # 1024 trajectories per core
SUBS, COLS = 4, 256

F32 = mybir.dt.float32
F32R = mybir.dt.float32r
F16 = mybir.dt.float16

_c1 = np.array([1, 1, -1, -1], np.float64)
_c2 = np.array([1, -1, 1, -1], np.float64)
_z = np.array([1, 0, 0, -1], np.float64)
_sx = np.array([[0, 1], [1, 0]], np.float64)
_HX = 0.5 * (np.kron(_sx, np.eye(2)) + np.kron(np.eye(2), _sx))

_A1 = (_c1[:, None] + _c1[None, :]).ravel()
_A2 = (_c2[:, None] + _c2[None, :]).ravel()
_D = (_c1[:, None] * _c1[None, :] + _c2[:, None] * _c2[None, :] - 2.0).ravel()
_Z = (_z[:, None] - _z[None, :]).ravel()

# MHX[c, c'] : (HX M - M HX)_c = sum_c' MHX[c,c'] M_c'
_MHX = np.zeros((16, 16), np.float64)
for _i in range(4):
    for _j in range(4):
        for _ii in range(4):
            for _jj in range(4):
                _v = _HX[_i, _ii] * (_j == _jj) - (_i == _ii) * _HX[_jj, _j]
                _MHX[4 * _i + _j, 4 * _ii + _jj] = _v


def _meas_ops():
    sx = np.array([[0, 1], [1, 0]], np.complex128)
    sy = np.array([[0, -1j], [1j, 0]], np.complex128)
    sz = np.array([[1, 0], [0, -1]], np.complex128)
    I2 = np.eye(2, dtype=np.complex128)
    paulis = [sx, sy, sz]
    ops = []
    for A in paulis:
        for Bm in paulis:
            for sa in (1, -1):
                for sb in (1, -1):
                    ops.append(np.kron((I2 + sa * A) / 2, (I2 + sb * Bm) / 2))
    for A in paulis:
        ops.append(np.kron((I2 + A) / 2, I2))
    for Bm in paulis:
        ops.append(np.kron(I2, (I2 + Bm) / 2))
    return np.stack(ops)  # [42, 4, 4]


_MEAS = _meas_ops()


def _part(xy, u, c):
    return xy * 64 + u * 16 + c


def _host_weights(eps, gamma):
    """Constant lhsT matrices. lhsT[k, m]: out[m,n] = sum_k lhsT[k,m]*rhs[k,n]."""
    zeps = np.zeros((128, 128), np.float64)
    mxm = np.zeros((128, 128), np.float64)
    for u in range(SUBS):
        for c in range(16):
            zeps[_part(1, u, c), _part(0, u, c)] = DT * eps * _Z[c]
            zeps[_part(0, u, c), _part(1, u, c)] = -DT * eps * _Z[c]
            for cp in range(16):
                if _MHX[c, cp] != 0.0:
                    mxm[_part(1, u, cp), _part(0, u, c)] = _MHX[c, cp]
                    mxm[_part(0, u, cp), _part(1, u, c)] = -_MHX[c, cp]
    fmat = np.zeros((128, 8), np.float64)
    for u in range(SUBS):
        for i in range(4):
            fmat[_part(0, u, 5 * i), u] = 2.0 * _c1[i]
            fmat[_part(0, u, 5 * i), 4 + u] = 2.0 * _c2[i]
    selgh = np.zeros((8, 128), np.float64)
    negsel = np.zeros((8, 128), np.float64)
    for xy in range(2):
        for u in range(SUBS):
            for c in range(16):
                m = _part(xy, u, c)
                selgh[u, m] = _A1[c]
                selgh[4 + u, m] = _A2[c]
                negsel[u, m] = -1.0
                negsel[4 + u, m] = -1.0
    wdg = np.zeros((128, 1), np.float64)
    for xy in range(2):
        for u in range(SUBS):
            for c in range(16):
                wdg[_part(xy, u, c), 0] = 1.0 + DT * gamma * _D[c]
    summ = np.zeros((128, 32), np.float64)
    for xy in range(2):
        for u in range(SUBS):
            for c in range(16):
                summ[_part(xy, u, c), xy * 16 + c] = 1.0
    fz = zeps @ fmat          # lhsT composition: (F o Zeps)[k, m']
    fm = mxm @ fmat
    f16, f32 = np.float16, np.float32
    return dict(
        zeps_w=zeps.astype(f16), mxm_w=mxm.astype(f16), f_w=fmat.astype(f16),
        fz_w=fz.astype(f16), fm_w=fm.astype(f16), selgh_w=selgh.astype(f16),
        negsel_w=negsel.astype(f16), wdg_w=wdg.astype(f32),
        summ_w=summ.astype(f32))


def _round_f32r(a):
    """Round float32 array to fp32r (11 mantissa bits, RNE) — what the PE
    expects of any tensor feeding a float32r matmul."""
    bits = np.ascontiguousarray(a, np.float32).view(np.uint32).copy()
    keep = (bits >> 12) & 1
    bits += 0x7FF + keep
    bits &= np.uint32(0xFFFFF000)
    return bits.view(np.float32)


_PROGRAM_CACHE = {}


def _build_program():
    if "nc" in _PROGRAM_CACHE:
        return _PROGRAM_CACHE["nc"]
    nc = bacc.Bacc("TRN2", target_bir_lowering=False, debug=False,
                   enable_asserts=False, num_devices=NCORES)

    noise_d = nc.dram_tensor("noise", [NSTEP, 8, COLS], F16, kind="ExternalInput")
    r0_d = nc.dram_tensor("r0", [128, COLS], F32, kind="ExternalInput")
    dtom_d = nc.dram_tensor("dtom", [128, COLS], F32, kind="ExternalInput")
    wdims = {"zeps_w": ([128, 128], F16), "mxm_w": ([128, 128], F16),
             "f_w": ([128, 8], F16), "fz_w": ([128, 8], F16),
             "fm_w": ([128, 8], F16), "selgh_w": ([8, 128], F16),
             "negsel_w": ([8, 128], F16), "wdg_w": ([128, 1], F32),
             "summ_w": ([128, 32], F32)}
    wdram = {n: nc.dram_tensor(n, shp, dt, kind="ExternalInput")
             for n, (shp, dt) in wdims.items()}
    out_d = nc.dram_tensor("partial", [32, B], F32, kind="ExternalOutput")

    MULT, ADD = AluOpType.mult, AluOpType.add

    with tile.TileContext(nc) as tc:
        with (
            tc.tile_pool(name="const", bufs=1) as cpool,
            tc.tile_pool(name="state", bufs=2) as spool,
            tc.tile_pool(name="work", bufs=2) as wpool,
            tc.tile_pool(name="gh", bufs=6) as ghpool,
            tc.tile_pool(name="psum", bufs=2, space="PSUM") as ppool,
        ):
            w = {}
            for n, (shp, dt) in wdims.items():
                w[n] = cpool.tile(shp, dt, tag=n, name=n + "_sb")
                nc.sync.dma_start(w[n][:], wdram[n].ap())
            dtom = cpool.tile([128, COLS], F32, tag="dtom")
            nc.sync.dma_start(dtom[:], dtom_d.ap())

            r = spool.tile([128, COLS], F32, tag="r")
            nc.sync.dma_start(r[:], r0_d.ap())
            r16 = spool.tile([128, COLS], F16, tag="r16")
            nc.scalar.copy(r16[:], r[:])
            qp16 = spool.tile([128, COLS], F16, tag="qp16")
            nc.gpsimd.tensor_mul(qp16[:], r[:], dtom[:])

            # step-0 prep: T(0) = F @ r(0) (full fp32), p8(0), W(0)
            gh = ghpool.tile([8, COLS], F16, tag="gh")
            nc.sync.dma_start(gh[:], noise_d.ap()[0, :, :])
            psum_t = ppool.tile([8, COLS], F32, tag="T")
            nc.tensor.matmul(psum_t[:], w["f_w"][:], r16[:], start=True, stop=True)
            p8 = wpool.tile([8, COLS], F16, tag="p8")
            nc.vector.tensor_tensor(p8[:], gh[:], psum_t[:], op=MULT)
            psum_w = ppool.tile([128, COLS], F32, tag="W")
            nc.tensor.matmul(psum_w[:], w["selgh_w"][:], gh[:],
                             start=True, stop=False)
            nc.tensor.matmul(psum_w[:], w["negsel_w"][:], p8[:],
                             start=False, stop=True)

            for k in range(NSTEP):
                last = k == NSTEP - 1
                if not last:
                    gh_nx = ghpool.tile([8, COLS], F16, tag="gh")
                    nc.sync.dma_start(gh_nx[:], noise_d.ap()[k + 1, :, :])

                psum_a = ppool.tile([128, COLS], F32, tag="A")
                nc.tensor.matmul(psum_a[:], w["zeps_w"][:], r16[:],
                                 start=True, stop=False)
                nc.tensor.matmul(psum_a[:], w["mxm_w"][:], qp16[:],
                                 start=False, stop=True)
                if not last:
                    psum_t_nx = ppool.tile([8, COLS], F32, tag="T")
                    nc.tensor.matmul(psum_t_nx[:], w["fz_w"][:], r16[:],
                                     start=True, stop=False)
                    nc.tensor.matmul(psum_t_nx[:], w["fm_w"][:], qp16[:],
                                     start=False, stop=False)

                v = wpool.tile([128, COLS], F32, tag="v")
                nc.vector.scalar_tensor_tensor(v[:], psum_w[:], w["wdg_w"][:],
                                               r[:], op0=ADD, op1=MULT)
                if not last:
                    v16 = wpool.tile([128, COLS], F16, tag="v16")
                    nc.vector.tensor_copy(v16[:], v[:])
                    nc.tensor.matmul(psum_t_nx[:], w["f_w"][:], v16[:],
                                     start=False, stop=True)
                r_nx = spool.tile([128, COLS], F32, tag="r")
                nc.vector.tensor_tensor(r_nx[:], v[:], psum_a[:], op=ADD)

                if not last:
                    r16_nx = spool.tile([128, COLS], F16, tag="r16")
                    nc.scalar.copy(r16_nx[:], r_nx[:])
                    qp16_nx = spool.tile([128, COLS], F16, tag="qp16")
                    nc.gpsimd.tensor_mul(qp16_nx[:], r_nx[:], dtom[:])
                    p8_nx = wpool.tile([8, COLS], F16, tag="p8")
                    nc.vector.tensor_tensor(p8_nx[:], gh_nx[:], psum_t_nx[:],
                                            op=MULT)
                    psum_w_nx = ppool.tile([128, COLS], F32, tag="W")
                    nc.tensor.matmul(psum_w_nx[:], w["selgh_w"][:], gh_nx[:],
                                     start=True, stop=False)
                    nc.tensor.matmul(psum_w_nx[:], w["negsel_w"][:], p8_nx[:],
                                     start=False, stop=True)
                    r, r16, qp16 = r_nx, r16_nx, qp16_nx
                    psum_w, psum_t = psum_w_nx, psum_t_nx
                else:
                    r = r_nx

            psum_o = ppool.tile([32, COLS], F32, tag="T")
            nc.tensor.matmul(psum_o[:], w["summ_w"][:], r[:],
                             start=True, stop=True)
            o1 = wpool.tile([32, COLS], F32, tag="o1")
            nc.vector.tensor_copy(o1[:], psum_o[:])
            o2 = wpool.tile([32, 128], F32, tag="o2")
            nc.vector.tensor_tensor(o2[:], o1[:, 0:128], o1[:, 128:256], op=ADD)
            o3 = wpool.tile([32, B], F32, tag="o3")
            nc.vector.tensor_tensor(o3[:], o2[:, 0:B], o2[:, B:128], op=ADD)
            nc.sync.dma_start(out_d.ap(), o3[:])

    nc.compile()
    _PROGRAM_CACHE["nc"] = nc
    return nc


def _host_prep(inputs, params, wvec, rho0):
    inputs = np.asarray(inputs, np.float64)
    params = np.asarray(params, np.float64)
    wvec64 = np.asarray(wvec, np.float64)
    rho0 = np.asarray(rho0)

    omega = inputs[:, 0] + 1e-8
    eps, gamma, eta = params[1], params[2], params[3]
    sqge = np.sqrt(max(gamma * eta, 0.0))

    weights = _host_weights(eps, gamma)

    dtom = np.tile((DT * omega).astype(np.float32)[None, :], (128, SUBS))
    # column n -> batch b = n % 64 (j = u*256 + n, 256 % 64 == 0)

    x0 = np.real(rho0).astype(np.float64).ravel()
    y0 = np.imag(rho0).astype(np.float64).ravel()
    r0 = np.zeros((128, COLS), np.float32)
    for u in range(SUBS):
        r0[_part(0, u, 0):_part(0, u, 0) + 16, :] = x0[:, None]
        r0[_part(1, u, 0):_part(1, u, 0) + 16, :] = y0[:, None]

    noise_cores = []
    for core in range(NCORES):
        wv = wvec64[core * NLOC:(core + 1) * NLOC]          # [1024, 255, 2]
        wv = (sqge * wv).astype(np.float16)
        wv = wv.reshape(SUBS, COLS, NSTEP, 2)                # [u, n, k, ch]
        noise = np.empty((NSTEP, 8, COLS), np.float16)
        noise[:, 0:4, :] = wv[:, :, :, 0].transpose(2, 0, 1)
        noise[:, 4:8, :] = wv[:, :, :, 1].transpose(2, 0, 1)
        noise_cores.append(np.ascontiguousarray(noise))

    shared = dict(weights, r0=r0, dtom=dtom)
    in_maps = [dict(shared, noise=noise_cores[c]) for c in range(NCORES)]
    return in_maps, inputs


def _postprocess(partials, inputs64):
    """partials: list of 8 arrays [32, 64] -> (out [64,43] f64, rho_mean c128)."""
    acc = np.zeros((32, B), np.float64)
    for p in partials:
        acc += np.asarray(p, np.float64)
    acc /= NT
    xbar = acc[0:16]            # [16, 64], c-major
    ybar = acc[16:32]
    rho_mean = (xbar + 1j * ybar).T.reshape(B, 4, 4)
    probs = np.einsum("kij,bji->bk", _MEAS, rho_mean).real
    probs = np.clip(probs, 0.0, 1.0)
    out = np.concatenate([probs, inputs64], axis=1)
    return out, rho_mean


TRACE = bool(int(os.environ.get("KERNEL_TRACE", "0")))
LAST_RESULTS = None


def kernel(inputs, params, wvec, rho0):
    global LAST_RESULTS
    in_maps, inputs64 = _host_prep(inputs, params, wvec, rho0)
    nc = _build_program()
    res = run_bass_kernel_spmd(nc, in_maps, core_ids=list(range(NCORES)),
                               trace=TRACE)
    LAST_RESULTS = res
    partials = [res.results[c]["partial"] for c in range(NCORES)]
    return _postprocess(partials, inputs64)


# revision 17
# speedup vs baseline: 1.4323x; 1.1815x over previous
"""Trainium2 Bass kernel for the two-qubit weak-measurement Euler SDE sampler.

Math reformulation (exactly equivalent to the reference, validated to 1e-15):
  rho = X + iY (Hermitian, X sym / Y antisym real 4x4), flattened c = 4*i+j.
  C1, C2, HZ are diagonal, so the dissipator / measurement / detuning terms are
  elementwise masks over c; only the HX commutator mixes components and is
  applied as a constant 32x32 matrix on the Omega-prescaled state qp = dt*Omega*r.

  Per Euler step (g = sqge*dw1, h = sqge*dw2, per trajectory):
    t1 = 2*(X00+X11-X22-X33), t2 = 2*(X00-X11+X22-X33)
    W  = g*A1 + h*A2 - (g*t1 + h*t2)          (noise multiplier, via PE)
    X' = X*(W + 1 + dt*G*D) + dt*(eps*Z.Y + Omega*MHX(Y))
    Y' = Y*(W + 1 + dt*G*D) - dt*(eps*Z.X + Omega*MHX(X))

Device layout (per core, 1024 trajectories = 16 traj-copies x 64 batch):
  partitions m = xy*64 + u*16 + c   (xy: 0=X 1=Y, u: sub-block of 256 traj)
  free n = 0..255; local traj j = u*256 + n; batch b = n % 64.

Sharding: trajectory-copy axis across 8 cores (t in [16k, 16k+16)), final mean
reduced on host.
"""

import os
import sys

for _p in ("/opt/trn_rl_repo", "/opt/pypackages"):
    if _p not in sys.path:
        sys.path.append(_p)

import numpy as np
import concourse.bass as bass
import concourse.bacc as bacc
import concourse.mybir as mybir
import concourse.tile as tile
from concourse.tile import add_dep_helper
from concourse.alu_op_type import AluOpType
from concourse.bass_utils import run_bass_kernel_spmd

B, NT, NSTEP = 64, 128, 255
DT = 2.0 ** -8
NCORES = 8
TPC = NT // NCORES        # 16 trajectory copies per core
NLOC = TPC * B            # 1024 trajectories per core
SUBS, COLS = 4, 256

F32 = mybir.dt.float32
F32R = mybir.dt.float32r
F16 = mybir.dt.float16

_c1 = np.array([1, 1, -1, -1], np.float64)
_c2 = np.array([1, -1, 1, -1], np.float64)
_z = np.array([1, 0, 0, -1], np.float64)
_sx = np.array([[0, 1], [1, 0]], np.float64)
_HX = 0.5 * (np.kron(_sx, np.eye(2)) + np.kron(np.eye(2), _sx))

_A1 = (_c1[:, None] + _c1[None, :]).ravel()
_A2 = (_c2[:, None] + _c2[None, :]).ravel()
_D = (_c1[:, None] * _c1[None, :] + _c2[:, None] * _c2[None, :] - 2.0).ravel()
_Z = (_z[:, None] - _z[None, :]).ravel()

# MHX[c, c'] : (HX M - M HX)_c = sum_c' MHX[c,c'] M_c'
_MHX = np.zeros((16, 16), np.float64)
for _i in range(4):
    for _j in range(4):
        for _ii in range(4):
            for _jj in range(4):
                _v = _HX[_i, _ii] * (_j == _jj) - (_i == _ii) * _HX[_jj, _j]
                _MHX[4 * _i + _j, 4 * _ii + _jj] = _v


def _meas_ops():
    sx = np.array([[0, 1], [1, 0]], np.complex128)
    sy = np.array([[0, -1j], [1j, 0]], np.complex128)
    sz = np.array([[1, 0], [0, -1]], np.complex128)
    I2 = np.eye(2, dtype=np.complex128)
    paulis = [sx, sy, sz]
    ops = []
    for A in paulis:
        for Bm in paulis:
            for sa in (1, -1):
                for sb in (1, -1):
                    ops.append(np.kron((I2 + sa * A) / 2, (I2 + sb * Bm) / 2))
    for A in paulis:
        ops.append(np.kron((I2 + A) / 2, I2))
    for Bm in paulis:
        ops.append(np.kron(I2, (I2 + Bm) / 2))
    return np.stack(ops)  # [42, 4, 4]


_MEAS = _meas_ops()


def _part(xy, u, c):
    return xy * 64 + u * 16 + c


def _host_weights(eps, gamma):
    """Constant lhsT matrices. lhsT[k, m]: out[m,n] = sum_k lhsT[k,m]*rhs[k,n]."""
    zeps = np.zeros((128, 128), np.float64)
    mxm = np.zeros((128, 128), np.float64)
    for u in range(SUBS):
        for c in range(16):
            zeps[_part(1, u, c), _part(0, u, c)] = DT * eps * _Z[c]
            zeps[_part(0, u, c), _part(1, u, c)] = -DT * eps * _Z[c]
            for cp in range(16):
                if _MHX[c, cp] != 0.0:
                    mxm[_part(1, u, cp), _part(0, u, c)] = _MHX[c, cp]
                    mxm[_part(0, u, cp), _part(1, u, c)] = -_MHX[c, cp]
    fmat = np.zeros((128, 8), np.float64)
    for u in range(SUBS):
        for i in range(4):
            fmat[_part(0, u, 5 * i), u] = 2.0 * _c1[i]
            fmat[_part(0, u, 5 * i), 4 + u] = 2.0 * _c2[i]
    selgh = np.zeros((8, 128), np.float64)
    negsel = np.zeros((8, 128), np.float64)
    for xy in range(2):
        for u in range(SUBS):
            for c in range(16):
                m = _part(xy, u, c)
                selgh[u, m] = _A1[c]
                selgh[4 + u, m] = _A2[c]
                negsel[u, m] = -1.0
                negsel[4 + u, m] = -1.0
    wdg = np.zeros((128, 1), np.float64)
    for xy in range(2):
        for u in range(SUBS):
            for c in range(16):
                wdg[_part(xy, u, c), 0] = 1.0 + DT * gamma * _D[c]
    summ = np.zeros((128, 32), np.float64)
    for xy in range(2):
        for u in range(SUBS):
            for c in range(16):
                summ[_part(xy, u, c), xy * 16 + c] = 1.0
    fz = zeps @ fmat          # lhsT composition: (F o Zeps)[k, m']
    fm = mxm @ fmat
    f16, f32 = np.float16, np.float32
    return dict(
        zeps_w=zeps.astype(f16), mxm_w=mxm.astype(f16), f_w=fmat.astype(f16),
        fz_w=fz.astype(f16), fm_w=fm.astype(f16), selgh_w=selgh.astype(f16),
        negsel_w=negsel.astype(f16), wdg_w=wdg.astype(f32),
        summ_w=summ.astype(f32))


def _round_f32r(a):
    """Round float32 array to fp32r (11 mantissa bits, RNE) — what the PE
    expects of any tensor feeding a float32r matmul."""
    bits = np.ascontiguousarray(a, np.float32).view(np.uint32).copy()
    keep = (bits >> 12) & 1
    bits += 0x7FF + keep
    bits &= np.uint32(0xFFFFF000)
    return bits.view(np.float32)


_PROGRAM_CACHE = {}


def _build_program():
    if "nc" in _PROGRAM_CACHE:
        return _PROGRAM_CACHE["nc"]
    nc = bacc.Bacc("TRN2", target_bir_lowering=False, debug=False,
                   enable_asserts=False, num_devices=NCORES)

    noise_d = nc.dram_tensor("noise", [NSTEP, 8, COLS], F16, kind="ExternalInput")
    r0_d = nc.dram_tensor("r0", [128, COLS], F32, kind="ExternalInput")
    dtom_d = nc.dram_tensor("dtom", [128, COLS], F32, kind="ExternalInput")
    wdims = {"zeps_w": ([128, 128], F16), "mxm_w": ([128, 128], F16),
             "f_w": ([128, 8], F16), "fz_w": ([128, 8], F16),
             "fm_w": ([128, 8], F16), "selgh_w": ([8, 128], F16),
             "negsel_w": ([8, 128], F16), "wdg_w": ([128, 1], F32),
             "summ_w": ([128, 32], F32)}
    wdram = {n: nc.dram_tensor(n, shp, dt, kind="ExternalInput")
             for n, (shp, dt) in wdims.items()}
    out_d = nc.dram_tensor("partial", [32, B], F32, kind="ExternalOutput")

    MULT, ADD = AluOpType.mult, AluOpType.add

    with tile.TileContext(nc) as tc:
        with (
            tc.tile_pool(name="const", bufs=1) as cpool,
            tc.tile_pool(name="state", bufs=2) as spool,
            tc.tile_pool(name="work", bufs=2) as wpool,
            tc.tile_pool(name="gh", bufs=6) as ghpool,
            tc.tile_pool(name="psum", bufs=2, space="PSUM") as ppool,
        ):
            w = {}
            for n, (shp, dt) in wdims.items():
                w[n] = cpool.tile(shp, dt, tag=n, name=n + "_sb")
                nc.sync.dma_start(w[n][:], wdram[n].ap())
            dtom = cpool.tile([128, COLS], F32, tag="dtom")
            nc.sync.dma_start(dtom[:], dtom_d.ap())

            r = spool.tile([128, COLS], F32, tag="r")
            nc.sync.dma_start(r[:], r0_d.ap())
            r16 = spool.tile([128, COLS], F16, tag="r16")
            nc.scalar.copy(r16[:], r[:])
            qp16 = spool.tile([128, COLS], F16, tag="qp16")
            nc.gpsimd.tensor_mul(qp16[:], r[:], dtom[:])

            # step-0 prep: T(0) = F @ r(0) (full fp32), p8(0), W(0)
            gh = ghpool.tile([8, COLS], F16, tag="gh")
            nc.sync.dma_start(gh[:], noise_d.ap()[0, :, :])
            psum_t = ppool.tile([8, COLS], F32, tag="T")
            nc.tensor.matmul(psum_t[:], w["f_w"][:], r16[:], start=True, stop=True)
            p8 = wpool.tile([8, COLS], F16, tag="p8")
            nc.vector.tensor_tensor(p8[:], gh[:], psum_t[:], op=MULT)
            psum_w = ppool.tile([128, COLS], F32, tag="W")
            nc.tensor.matmul(psum_w[:], w["selgh_w"][:], gh[:],
                             start=True, stop=False)
            nc.tensor.matmul(psum_w[:], w["negsel_w"][:], p8[:],
                             start=False, stop=True)

            for k in range(NSTEP):
                last = k == NSTEP - 1
                if not last:
                    gh_nx = ghpool.tile([8, COLS], F16, tag="gh")
                    nc.sync.dma_start(gh_nx[:], noise_d.ap()[k + 1, :, :])

                psum_a = ppool.tile([128, COLS], F32, tag="A")
                nc.tensor.matmul(psum_a[:], w["zeps_w"][:], r16[:],
                                 start=True, stop=False)
                nc.tensor.matmul(psum_a[:], w["mxm_w"][:], qp16[:],
                                 start=False, stop=True)
                if not last:
                    psum_t_nx = ppool.tile([8, COLS], F32, tag="T")
                    nc.tensor.matmul(psum_t_nx[:], w["fz_w"][:], r16[:],
                                     start=True, stop=False)
                    nc.tensor.matmul(psum_t_nx[:], w["fm_w"][:], qp16[:],
                                     start=False, stop=False)

                v = wpool.tile([128, COLS], F32, tag="v")
                nc.vector.scalar_tensor_tensor(v[:], psum_w[:], w["wdg_w"][:],
                                               r[:], op0=ADD, op1=MULT)
                if not last:
                    v16 = wpool.tile([128, COLS], F16, tag="v16")
                    nc.vector.tensor_copy(v16[:], v[:])
                    i_mmt1 = nc.tensor.matmul(psum_t_nx[:], w["f_w"][:], v16[:],
                                              start=False, stop=True)
                r_nx = spool.tile([128, COLS], F32, tag="r")
                nc.vector.tensor_tensor(r_nx[:], v[:], psum_a[:], op=ADD)

                if not last:
                    r16_nx = spool.tile([128, COLS], F16, tag="r16")
                    nc.scalar.copy(r16_nx[:], r_nx[:])
                    qp16_nx = spool.tile([128, COLS], F16, tag="qp16")
                    nc.gpsimd.tensor_mul(qp16_nx[:], r_nx[:], dtom[:])
                    p8_nx = wpool.tile([8, COLS], F16, tag="p8")
                    nc.vector.tensor_tensor(p8_nx[:], gh_nx[:], psum_t_nx[:],
                                            op=MULT)
                    psum_w_nx = ppool.tile([128, COLS], F32, tag="W")
                    i_mm3 = nc.tensor.matmul(psum_w_nx[:], w["selgh_w"][:],
                                             gh_nx[:], start=True, stop=False)
                    add_dep_helper(i_mm3.ins, i_mmt1.ins, sync=False,
                                   reason="keep chain mm_t1 ahead of mm3 on PE")
                    nc.tensor.matmul(psum_w_nx[:], w["negsel_w"][:], p8_nx[:],
                                     start=False, stop=True)
                    r, r16, qp16 = r_nx, r16_nx, qp16_nx
                    psum_w, psum_t = psum_w_nx, psum_t_nx
                else:
                    r = r_nx

            psum_o = ppool.tile([32, COLS], F32, tag="T")
            nc.tensor.matmul(psum_o[:], w["summ_w"][:], r[:],
                             start=True, stop=True)
            o1 = wpool.tile([32, COLS], F32, tag="o1")
            nc.vector.tensor_copy(o1[:], psum_o[:])
            o2 = wpool.tile([32, 128], F32, tag="o2")
            nc.vector.tensor_tensor(o2[:], o1[:, 0:128], o1[:, 128:256], op=ADD)
            o3 = wpool.tile([32, B], F32, tag="o3")
            nc.vector.tensor_tensor(o3[:], o2[:, 0:B], o2[:, B:128], op=ADD)
            nc.sync.dma_start(out_d.ap(), o3[:])

    nc.compile()
    _PROGRAM_CACHE["nc"] = nc
    return nc


def _host_prep(inputs, params, wvec, rho0):
    inputs = np.asarray(inputs, np.float64)
    params = np.asarray(params, np.float64)
    wvec64 = np.asarray(wvec, np.float64)
    rho0 = np.asarray(rho0)

    omega = inputs[:, 0] + 1e-8
    eps, gamma, eta = params[1], params[2], params[3]
    sqge = np.sqrt(max(gamma * eta, 0.0))

    weights = _host_weights(eps, gamma)

    dtom = np.tile((DT * omega).astype(np.float32)[None, :], (128, SUBS))
    # column n -> batch b = n % 64 (j = u*256 + n, 256 % 64 == 0)

    x0 = np.real(rho0).astype(np.float64).ravel()
    y0 = np.imag(rho0).astype(np.float64).ravel()
    r0 = np.zeros((128, COLS), np.float32)
    for u in range(SUBS):
        r0[_part(0, u, 0):_part(0, u, 0) + 16, :] = x0[:, None]
        r0[_part(1, u, 0):_part(1, u, 0) + 16, :] = y0[:, None]

    noise_cores = []
    for core in range(NCORES):
        wv = wvec64[core * NLOC:(core + 1) * NLOC]          # [1024, 255, 2]
        wv = (sqge * wv).astype(np.float16)
        wv = wv.reshape(SUBS, COLS, NSTEP, 2)                # [u, n, k, ch]
        noise = np.empty((NSTEP, 8, COLS), np.float16)
        noise[:, 0:4, :] = wv[:, :, :, 0].transpose(2, 0, 1)
        noise[:, 4:8, :] = wv[:, :, :, 1].transpose(2, 0, 1)
        noise_cores.append(np.ascontiguousarray(noise))

    shared = dict(weights, r0=r0, dtom=dtom)
    in_maps = [dict(shared, noise=noise_cores[c]) for c in range(NCORES)]
    return in_maps, inputs


def _postprocess(partials, inputs64):
    """partials: list of 8 arrays [32, 64] -> (out [64,43] f64, rho_mean c128)."""
    acc = np.zeros((32, B), np.float64)
    for p in partials:
        acc += np.asarray(p, np.float64)
    acc /= NT
    xbar = acc[0:16]            # [16, 64], c-major
    ybar = acc[16:32]
    rho_mean = (xbar + 1j * ybar).T.reshape(B, 4, 4)
    probs = np.einsum("kij,bji->bk", _MEAS, rho_mean).real
    probs = np.clip(probs, 0.0, 1.0)
    out = np.concatenate([probs, inputs64], axis=1)
    return out, rho_mean


TRACE = bool(int(os.environ.get("KERNEL_TRACE", "0")))
LAST_RESULTS = None


def kernel(inputs, params, wvec, rho0):
    global LAST_RESULTS
    in_maps, inputs64 = _host_prep(inputs, params, wvec, rho0)
    nc = _build_program()
    res = run_bass_kernel_spmd(nc, in_maps, core_ids=list(range(NCORES)),
                               trace=TRACE)
    LAST_RESULTS = res
    partials = [res.results[c]["partial"] for c in range(NCORES)]
    return _postprocess(partials, inputs64)
